# revision 20
# baseline (speedup 1.0000x reference)
# Trainium2 Bass kernel for nn_AdaptiveEmbedding (8 NeuronCores, SPMD).
#
# Reference math (per full batch B=256, R=36, T=64, D=1024):
#   cap = l2norm(leaky_relu(cap_embed^T), axis=T); masked mean over t<len -> cap_mean (B,D)
#   img = l2norm(leaky_relu(img_embed^T), axis=R); BN1d (batch stats) -> base = bn.mean(R)
#   alphas/betas = fc(cap_mean) interleaved; img_vec = l2norm(base*alpha+beta, D)
#   sims[b,c] = <img_vec[c,b], l2norm(cap_mean)[c]>
#
# Algebraic reduction used here (never materialize (C,B,D)):
#   num[b,c]  = sum_d base[b,d]*(alpha*cap_mean)[c,d] + cb[c],  cb[c]=sum_d beta*cap_mean
#   den2[b,c] = sum_d base^2[b,d]*alpha^2[c,d] + sum_d base[b,d]*(2*alpha*beta)[c,d] + q3[c]
#   sims[b,c] = invn[c] * num / (sqrt(den2) + 1e-8),  invn[c]=1/(||cap_mean[c]||+1e-8)
#
# Sharding: core k owns caps [32k,32k+32) and imgs [32k,32k+32).
#   - BN batch stats: 8KB AllReduce of per-core partial sums.
#   - base vectors: AllGather of (128,256) folded base^T -> (1024,256).
#   - sims columns: each core outputs its (256,32) slice; host concatenates.

import numpy as np
import ml_dtypes

B, R, T, D = 256, 36, 64, 1024
NCORES = 8
CLOC = B // NCORES  # 32 local captions / images
NPAIR = CLOC // 2  # 16 caption pair-tiles (2*64 tokens = 128 partitions)
NIT = 11  # image tiles of 3 imgs (108 partitions); last tile has 2 + zero pad
EPS_L2 = 1e-8
EPS_BN = 1e-5
LEAK = 0.1
F32 = None  # set in _build (mybir.dt.float32)

_STATE = {}


def _build(tap=None):
    import concourse.bass as bass
    import concourse.bacc as bacc
    import concourse.tile as tile
    from concourse import mybir

    f32 = mybir.dt.float32
    f32r = mybir.dt.float32r
    bf16 = mybir.dt.bfloat16
    AF = mybir.ActivationFunctionType
    ALU = mybir.AluOpType

    nc = bacc.Bacc(
        "TRN2",
        target_bir_lowering=False,
        debug=False,
        enable_asserts=True,
        num_devices=NCORES,
    )

    # ---- kernel I/O -----------------------------------------------------
    cap_in = nc.dram_tensor("cap", [NPAIR, 128, D], f32, kind="ExternalInput").ap()
    img_in = nc.dram_tensor("img", [NIT, 108, D], f32, kind="ExternalInput").ap()
    wm_in = nc.dram_tensor("wm2", [128, NPAIR, 64], f32r, kind="ExternalInput").ap()
    imo_in = nc.dram_tensor("imones", [108, NIT, 32], f32r, kind="ExternalInput").ap()
    fc_in = nc.dram_tensor("fcT", [2, 8, 128, 1024], bf16, kind="ExternalInput").ap()
    fcb_in = nc.dram_tensor("fcb", [128, 16], f32, kind="ExternalInput").ap()
    bnf_in = nc.dram_tensor("bnF", [128, 16], f32, kind="ExternalInput").ap()
    cst_in = nc.dram_tensor("consts", [128, 8], f32, kind="ExternalInput").ap()
    onesr_in = nc.dram_tensor("ones_row", [1, 128], f32, kind="ExternalInput").ap()
    id_in = nc.dram_tensor("ident", [32, 32], f32, kind="ExternalInput").ap()
    out = nc.dram_tensor("out", [B, CLOC], f32, kind="ExternalOutput").ap()

    _dbg = {}

    def tap_point(name, ap):
        if tap == name:
            shape = [ap.shape[0], int(np.prod(ap.shape[1:]))]
            dbg = nc.dram_tensor("dbg", shape, ap.dtype, kind="ExternalOutput").ap()
            nc.sync.dma_start(out=dbg[:, :], in_=ap)
            _dbg["done"] = True

    RG = [list(range(NCORES))]

    with tile.TileContext(nc) as tc:
        with (
            tc.tile_pool(name="dram", bufs=1, space="DRAM") as dpool,
            tc.tile_pool(name="io", bufs=3) as io,       # streaming input tiles
            tc.tile_pool(name="work", bufs=2) as work,   # relu/square working tiles
            tc.tile_pool(name="sb1", bufs=1) as sb1,     # long-lived single tensors
            tc.tile_pool(name="psA", bufs=1, space="PSUM") as psA,   # 4-bank accum
            tc.tile_pool(name="psS", bufs=4, space="PSUM") as psS,   # 1-bank smalls
        ):
            # ---- DRAM bounce buffers for collectives ----
            ar_in = dpool.tile([128, 16], f32)
            ar_out = dpool.tile([128, 16], f32, addr_space="Shared")
            ag_in = dpool.tile([128, 256], f32)
            ag_out = dpool.tile([1024, 256], f32, addr_space="Shared")

            # ---- constants ----
            csts = sb1.tile([128, 8], f32)
            nc.gpsimd.dma_start(out=csts[:], in_=cst_in[:, :])
            ones_row = sb1.tile([1, 128], f32)
            nc.gpsimd.dma_start(out=ones_row[:], in_=onesr_in[:, :])
            ident = sb1.tile([32, 32], f32)
            nc.gpsimd.dma_start(out=ident[:], in_=id_in[:, :])
            wm_sb = sb1.tile([128, NPAIR, 64], f32r)
            nc.gpsimd.dma_start(out=wm_sb[:], in_=wm_in[:, :, :])
            imo_sb = sb1.tile([108, NIT, 32], f32r)
            nc.gpsimd.dma_start(out=imo_sb[:], in_=imo_in[:, :, :])
            fcb_sb = sb1.tile([128, 16], f32)
            nc.gpsimd.dma_start(out=fcb_sb[:], in_=fcb_in[:, :])
            bnf_sb = sb1.tile([128, 16], f32)
            nc.gpsimd.dma_start(out=bnf_sb[:], in_=bnf_in[:, :])

            ones128 = csts[:, 5:6]     # ones column (128,1)

            def t_fold(dst_folded, src_sb, ncol=8):
                """(32, ncol*128) SBUF -> folded (128, ncol*32) via PE transpose."""
                for j in range(ncol):
                    pt = psS.tile([128, 32], f32, tag="sm")
                    nc.tensor.transpose(
                        pt[:], src_sb[:, j * 128 : (j + 1) * 128], ident[:]
                    )
                    nc.scalar.copy(dst_folded[:, j * 32 : (j + 1) * 32], pt[:])

            # ================= image pipeline (first: feeds collectives) ====
            img_acc = psA.tile([32, 2048], f32, tag="acc")  # [s1 | s2]
            for p in range(NIT):
                xt = io.tile([108, D], f32, tag="io")
                nc.sync.dma_start(out=xt[:], in_=img_in[p, :, :])
                yt = work.tile([108, D], f32r, tag="y")
                nc.vector.scalar_tensor_tensor(
                    yt[:], xt[:], LEAK, xt[:], ALU.mult, ALU.max
                )
                y2 = work.tile([108, D], f32r, tag="y2")
                nc.scalar.activation(y2[:], yt[:], AF.Square)
                st, sp = (p == 0), (p == NIT - 1)
                for h in range(2):
                    sl = slice(512 * h, 512 * (h + 1))
                    nc.tensor.matmul(
                        img_acc[:, sl],
                        lhsT=imo_sb[:, p, :],
                        rhs=yt[:, sl],
                        start=st,
                        stop=sp,
                    )
                    sl2 = slice(1024 + 512 * h, 1024 + 512 * (h + 1))
                    nc.tensor.matmul(
                        img_acc[:, sl2],
                        lhsT=imo_sb[:, p, :],
                        rhs=y2[:, sl],
                        start=st,
                        stop=sp,
                    )

            s1_sb = work.tile([32, D], f32, tag="cm1")
            nc.scalar.copy(s1_sb[:], img_acc[0:32, 0:1024])
            s2_sb = work.tile([32, D], f32, tag="cm2")
            nc.scalar.copy(s2_sb[:], img_acc[0:32, 1024:2048])

            s1T = sb1.tile([128, 256], f32)
            t_fold(s1T, s1_sb)
            s2T = sb1.tile([128, 256], f32)
            t_fold(s2T, s2_sb)

            # folded per-(b,d) norm math: n = sqrt(s2)+eps, w = 1/n
            nT = work.tile([128, 256], f32, tag="f1")
            nc.scalar.activation(nT[:], s2T[:], AF.Sqrt)
            nc.vector.tensor_scalar_add(nT[:], nT[:], EPS_L2)
            wT = work.tile([128, 256], f32, tag="f2")
            nc.vector.reciprocal(wT[:], nT[:])
            s1nT = sb1.tile([128, 256], f32)  # sum_r ytilde  (folded d-major)
            nc.vector.tensor_mul(s1nT[:], s1T[:], wT[:])
            # s2n = ((n-eps)/n)^2 = (1 - eps*w)^2
            tT = work.tile([128, 256], f32, tag="f3")
            nc.vector.tensor_scalar(tT[:], wT[:], -EPS_L2, 1.0, ALU.mult, ALU.add)
            s2nT = work.tile([128, 256], f32, tag="f4")
            nc.vector.tensor_mul(s2nT[:], tT[:], tT[:])
            tap_point("s1nT", s1nT[:])

            # partial batch sums over local b (free axis within folded chunks)
            stat = work.tile([128, 16], f32, tag="st")
            nc.vector.reduce_sum(
                stat[:, 0:8],
                s1nT[:].rearrange("p (j c) -> p j c", j=8),
                axis=mybir.AxisListType.X,
            )
            nc.vector.reduce_sum(
                stat[:, 8:16],
                s2nT[:].rearrange("p (j c) -> p j c", j=8),
                axis=mybir.AxisListType.X,
            )
            nc.gpsimd.dma_start(out=ar_in[:], in_=stat[:])
            nc.gpsimd.collective_compute(
                "AllReduce",
                ALU.add,
                replica_groups=RG,
                ins=[ar_in[:]],
                outs=[ar_out[:]],
            )
            statg = work.tile([128, 16], f32, tag="stg")
            nc.gpsimd.dma_start(out=statg[:], in_=ar_out[:])

            # BN stats (folded (128,8) columns): mu, var, g
            inv_br = 1.0 / (B * R)
            muF = work.tile([128, 8], f32, tag="s1")
            nc.vector.tensor_scalar_mul(muF[:], statg[:, 0:8], inv_br)
            varF = work.tile([128, 8], f32, tag="s2")
            # var = E2 - mu^2 = s2g/BR - mu*mu
            nc.vector.tensor_scalar_mul(varF[:], statg[:, 8:16], inv_br)
            mu2F = work.tile([128, 8], f32, tag="s3")
            nc.vector.tensor_mul(mu2F[:], muF[:], muF[:])
            nc.vector.tensor_sub(varF[:], varF[:], mu2F[:])
            nc.vector.tensor_scalar_add(varF[:], varF[:], EPS_BN)
            sqF = work.tile([128, 8], f32, tag="s4")
            nc.scalar.activation(sqF[:], varF[:], AF.Sqrt)
            gF = work.tile([128, 8], f32, tag="s5")
            nc.vector.reciprocal(gF[:], sqF[:])
            nc.vector.tensor_mul(gF[:], gF[:], bnf_sb[:, 0:8])  # g = bn_w/sqrt(var+e)
            GF = work.tile([128, 8], f32, tag="s6")
            nc.vector.tensor_scalar_mul(GF[:], gF[:], 1.0 / R)  # G = g/R
            HF = work.tile([128, 8], f32, tag="s7")
            # H = mu*g - bn_b
            nc.vector.tensor_mul(HF[:], muF[:], gF[:])
            nc.vector.tensor_sub(HF[:], HF[:], bnf_sb[:, 8:16])

            # baseT folded: base = s1n*G - H  (per-chunk tensor_scalar trick)
            baseT = sb1.tile([128, 256], f32)
            for j in range(8):
                nc.vector.tensor_scalar(
                    baseT[:, j * 32 : (j + 1) * 32],
                    s1nT[:, j * 32 : (j + 1) * 32],
                    GF[:, j : j + 1],
                    HF[:, j : j + 1],
                    ALU.mult,
                    ALU.subtract,
                )
            tap_point("baseT", baseT[:])
            nc.gpsimd.dma_start(out=ag_in[:], in_=baseT[:])
            nc.gpsimd.collective_compute(
                "AllGather",
                ALU.bypass,
                replica_groups=RG,
                ins=[ag_in[:]],
                outs=[ag_out[:]],
            )

            # ================= caption pipeline =============================
            cap_acc = psA.tile([32, 2048], f32, tag="acc")  # [wsum | sumsq]
            for p in range(NPAIR):
                xt = io.tile([128, D], f32, tag="io")
                nc.sync.dma_start(out=xt[:], in_=cap_in[p, :, :])
                yt = work.tile([128, D], f32r, tag="y")
                nc.vector.scalar_tensor_tensor(
                    yt[:], xt[:], LEAK, xt[:], ALU.mult, ALU.max
                )
                y2 = work.tile([128, D], f32r, tag="y2")
                nc.scalar.activation(y2[:], yt[:], AF.Square)
                st, sp = (p == 0), (p == NPAIR - 1)
                for h in range(2):
                    sl = slice(512 * h, 512 * (h + 1))
                    nc.tensor.matmul(
                        cap_acc[:, sl],
                        lhsT=wm_sb[:, p, 0:32],
                        rhs=yt[:, sl],
                        start=st,
                        stop=sp,
                    )
                    sl2 = slice(1024 + 512 * h, 1024 + 512 * (h + 1))
                    nc.tensor.matmul(
                        cap_acc[:, sl2],
                        lhsT=wm_sb[:, p, 32:64],
                        rhs=y2[:, sl],
                        start=st,
                        stop=sp,
                    )

            ws_sb = work.tile([32, D], f32, tag="cm1")
            nc.scalar.copy(ws_sb[:], cap_acc[0:32, 0:1024])
            sq_sb = work.tile([32, D], f32, tag="cm2")
            nc.scalar.copy(sq_sb[:], cap_acc[0:32, 1024:2048])
            tap_point("ws", ws_sb[:])
            tap_point("sq", sq_sb[:])
            wsT = sb1.tile([128, 256], f32)
            t_fold(wsT, ws_sb)
            sqT = sb1.tile([128, 256], f32)
            t_fold(sqT, sq_sb)

            # cap_mean = wsum / (sqrt(sumsq)+eps)   (folded)
            cnT = work.tile([128, 256], f32, tag="f1")
            nc.scalar.activation(cnT[:], sqT[:], AF.Sqrt)
            nc.vector.tensor_scalar_add(cnT[:], cnT[:], EPS_L2)
            ciT = work.tile([128, 256], f32, tag="f2")
            nc.vector.reciprocal(ciT[:], cnT[:])
            cmT = sb1.tile([128, 256], f32)
            nc.vector.tensor_mul(cmT[:], wsT[:], ciT[:])
            tap_point("cmT", cmT[:])
            cmTb = sb1.tile([128, 256], bf16)
            nc.vector.tensor_copy(cmTb[:], cmT[:])

            # ================= FC: alphas/betas (d-major folded) ============
            alT = sb1.tile([128, 256], f32)
            beT = sb1.tile([128, 256], f32)
            abT = [alT, beT]
            for a in range(2):
                for j in range(8):
                    fw = io.tile([128, 1024], bf16, tag="fw")
                    nc.scalar.dma_start(out=fw[:], in_=fc_in[a, j, :, :])
                    ps = psS.tile([128, 32], f32, tag="sm")
                    for i in range(8):
                        nc.tensor.matmul(
                            ps[:],
                            lhsT=fw[:, i * 128 : (i + 1) * 128],
                            rhs=cmTb[:, i * 32 : (i + 1) * 32],
                            start=(i == 0),
                            stop=(i == 7),
                        )
                    nc.vector.tensor_scalar_add(
                        abT[a][:, j * 32 : (j + 1) * 32],
                        ps[:],
                        fcb_sb[:, a * 8 + j : a * 8 + j + 1],
                    )

            tap_point("alT", alT[:])
            tap_point("beT", beT[:])
            # ================= per-caption folded products ==================
            na_rhs = sb1.tile([128, 8, 64], f32)  # [:,i,0:32]=uT, [:,i,32:64]=2ab
            a2T = sb1.tile([128, 256], f32)
            bcmT = work.tile([128, 256], f32, tag="f3")
            b2T = work.tile([128, 256], f32, tag="f4")
            cm2T = work.tile([128, 256], f32, tag="f5")
            nc.vector.tensor_mul(a2T[:], alT[:], alT[:])
            nc.vector.tensor_mul(bcmT[:], beT[:], cmT[:])
            nc.vector.tensor_mul(b2T[:], beT[:], beT[:])
            nc.vector.tensor_mul(cm2T[:], cmT[:], cmT[:])
            for i in range(8):
                sl = slice(i * 32, (i + 1) * 32)
                nc.vector.tensor_mul(na_rhs[:, i, 0:32], alT[:, sl], cmT[:, sl])
                tmp = na_rhs[:, i, 32:64]
                nc.vector.tensor_mul(tmp, alT[:, sl], beT[:, sl])
                nc.vector.tensor_add(tmp, tmp, tmp)

            # row reductions: cb, q3, nrm2 (each (1,32)) via ones-matmuls
            rows_cb = psS.tile([1, 32], f32, tag="sm")
            rows_q3 = psS.tile([1, 32], f32, tag="sm")
            rows_n2 = psS.tile([1, 32], f32, tag="sm")
            for i in range(8):
                sl = slice(i * 32, (i + 1) * 32)
                st, sp = (i == 0), (i == 7)
                nc.tensor.matmul(
                    rows_cb[:], lhsT=ones128, rhs=bcmT[:, sl], start=st, stop=sp
                )
                nc.tensor.matmul(
                    rows_q3[:], lhsT=ones128, rhs=b2T[:, sl], start=st, stop=sp
                )
                nc.tensor.matmul(
                    rows_n2[:], lhsT=ones128, rhs=cm2T[:, sl], start=st, stop=sp
                )
            tap_point("statg", statg[:])
            rows_sb = sb1.tile([1, 96], f32)
            nc.scalar.copy(rows_sb[:, 0:32], rows_cb[:])
            nc.scalar.copy(rows_sb[:, 32:64], rows_q3[:])
            nc.scalar.copy(rows_sb[:, 64:96], rows_n2[:])
            tap_point("narhs", na_rhs[:, :, :])
            tap_point("a2T", a2T[:])
            tap_point("rows", rows_sb[:])
            # invn = 1/(sqrt(nrm2)+eps)
            invn = sb1.tile([1, 32], f32)
            nc.scalar.activation(invn[:], rows_sb[:, 64:96], AF.Sqrt)
            nc.vector.tensor_scalar_add(invn[:], invn[:], EPS_L2)
            nc.vector.reciprocal(invn[:], invn[:])
            # broadcast invn across partitions via K=1 matmul
            inb_ps = psS.tile([128, 32], f32, tag="sm")
            nc.tensor.matmul(
                inb_ps[:], lhsT=ones_row[:],
                rhs=invn[:], start=True, stop=True,
            )
            invn_sb = sb1.tile([128, 32], f32)
            nc.scalar.copy(invn_sb[:], inb_ps[:])

            # ================= sims matmuls + epilogue ======================
            for m in range(2):  # img blocks of 128
                bsl = slice(m * 128, (m + 1) * 128)
                na = psS.tile([128, 64], f32, tag="sm")
                for i in range(8):
                    bT = io.tile([128, 256], f32, tag="bT")
                    # gather chunk i of global baseT from AG output
                    src = bass.AP(
                        tensor=ag_out.tensor,
                        offset=ag_out.offset + 32 * i,
                        ap=[[256, 128], [128 * 256, 8], [1, 32]],
                    )
                    nc.scalar.dma_start(out=bT[:], in_=src)
                    b2 = io.tile([128, 128], f32, tag="b2")
                    nc.vector.tensor_mul(b2[:], bT[:, bsl], bT[:, bsl])
                    nc.tensor.matmul(
                        na[:],
                        lhsT=bT[:, bsl],
                        rhs=na_rhs[:, i, :],
                        start=(i == 0),
                        stop=False,
                    )
                    nc.tensor.matmul(
                        na[:, 32:64],
                        lhsT=b2[:],
                        rhs=a2T[:, i * 32 : (i + 1) * 32],
                        start=False,
                        stop=False,
                        skip_group_check=True,
                    )
                # num += cb, den2 += q3 broadcast over b: K=1 matmul
                nc.tensor.matmul(
                    na[:],
                    lhsT=ones_row[:],
                    rhs=rows_sb[:, 0:64],
                    start=False,
                    stop=True,
                )
                if tap == "na0" and m == 0:
                    dbgn = nc.dram_tensor("dbg", [128, 64], f32, kind="ExternalOutput").ap()
                    nc.sync.dma_start(out=dbgn[:, :], in_=na[:, :])
                den = work.tile([128, 32], f32, tag="ep1")
                nc.scalar.activation(den[:], na[:, 32:64], AF.Sqrt)
                nc.vector.tensor_scalar_add(den[:], den[:], EPS_L2)
                rec = work.tile([128, 32], f32, tag="ep2")
                nc.vector.reciprocal(rec[:], den[:])
                sims = work.tile([128, 32], f32, tag="ep3")
                nc.vector.tensor_mul(sims[:], na[:, 0:32], rec[:])
                nc.vector.tensor_mul(sims[:], sims[:], invn_sb[:])
                nc.sync.dma_start(out=out[bsl, :], in_=sims[:])
            if tap == "agout":
                dbg = nc.dram_tensor("dbg", [1024, 256], f32, kind="ExternalOutput").ap()
                for jj in range(8):
                    agt = io.tile([128, 256], f32, tag="agt")
                    nc.sync.dma_start(out=agt[:], in_=ag_out[128 * jj : 128 * (jj + 1), :])
                    nc.sync.dma_start(out=dbg[128 * jj : 128 * (jj + 1), :], in_=agt[:])

    nc.compile()
    return nc


def _prep_inputs(img_embed, cap_embed, lens, fc_w, fc_b, bn_w, bn_b):
    f32 = np.float32
    bf16 = ml_dtypes.bfloat16
    lens_f = lens.astype(f32)
    wmask_w = (np.arange(T)[None, :] < lens[:, None]).astype(f32) / lens_f[:, None]

    # constants (same all cores)
    consts = np.zeros((128, 8), f32)
    consts[:, 5] = 1.0
    ones_row = np.ones((1, 128), f32)
    ident = np.eye(32, dtype=f32)

    # image block-diag ones lhsT per tile: col 3t+j = 1 on rows 36j:36(j+1)
    imones = np.zeros((108, NIT, 32), f32)
    for t in range(NIT):
        for j in range(min(3, CLOC - 3 * t)):
            imones[36 * j : 36 * (j + 1), t, 3 * t + j] = 1.0

    # fcT[a, j, kk, i*128+dd] = fc_w[2*(128j+dd)+a, 128i+kk]
    A = fc_w.reshape(1024, 2, 1024).transpose(1, 0, 2)  # (a, dout, k)
    A5 = A.reshape(2, 8, 128, 8, 128)  # (a, j, dd, i, kk)
    fcT = np.ascontiguousarray(A5.transpose(0, 1, 4, 3, 2)).reshape(2, 8, 128, 1024)
    fcT = fcT.astype(bf16)
    # fcb[dd, a*8+j] = fc_b[2*(128j+dd)+a]
    fcb = np.ascontiguousarray(
        fc_b.reshape(8, 128, 2).transpose(1, 2, 0)
    ).reshape(128, 16).astype(f32)
    # bnF[p, j]=bn_w[128j+p], bnF[p, 8+j]=bn_b[128j+p]
    bnF = np.concatenate(
        [bn_w.reshape(8, 128).T, bn_b.reshape(8, 128).T], axis=1
    ).astype(f32)

    in_maps = []
    for k in range(NCORES):
        s = slice(CLOC * k, CLOC * (k + 1))
        cap_k = np.ascontiguousarray(cap_embed[s]).reshape(NPAIR, 128, D)
        img_k = np.zeros((NIT, 108, D), f32)
        imgs = img_embed[s]
        for t in range(NIT):
            n = min(3, CLOC - 3 * t)
            img_k[t, : 36 * n, :] = imgs[3 * t : 3 * t + n].reshape(36 * n, D)
        wm = np.zeros((128, NPAIR, 64), f32)
        for p in range(NPAIR):
            for c in range(2):
                rows = slice(64 * c, 64 * (c + 1))
                wm[rows, p, 2 * p + c] = wmask_w[CLOC * k + 2 * p + c]
                wm[rows, p, 32 + 2 * p + c] = 1.0
        in_maps.append(
            {
                "cap": cap_k.astype(f32),
                "img": img_k,
                "wm2": wm,
                "imones": imones,
                "fcT": fcT,
                "fcb": fcb,
                "bnF": bnF,
                "consts": consts,
                "ones_row": ones_row,
                "ident": ident,
            }
        )
    return in_maps


def run(inputs, trace=False, tap=None, **kw):
    from concourse import bass_utils

    key = ("nc", tap)
    if key not in _STATE:
        _STATE[key] = _build(tap)
    _STATE["nc"] = _STATE[key]
    in_maps = _prep_inputs(**inputs)
    res = bass_utils.run_bass_kernel_spmd(
        _STATE["nc"], in_maps, core_ids=list(range(NCORES)), trace=trace, **kw
    )
    sims = np.concatenate([res.results[k]["out"] for k in range(NCORES)], axis=1)
    return sims.astype(np.float32), res


def kernel(**inputs):
    sims, _ = run(inputs, trace=False)
    return sims


# revision 21
# speedup vs baseline: 1.9371x; 1.9371x over previous
# Trainium2 Bass kernel for nn_AdaptiveEmbedding (8 NeuronCores, SPMD).
#
# Math (B=256, R=36, T=64, D=1024): see reference. Algebraic reduction:
#   num[b,c]  = sum_d base[b,d]*(alpha*cm)[c,d] + cb[c]
#   den2[b,c] = sum_d base^2*alpha^2 + sum_d base*(2*alpha*beta) + q3[c]
#   sims[b,c] = invn[c] * num / (sqrt(den2) + 1e-8)
# Sharding: core k owns caps/imgs [32k,32k+32). BN stats via 8KB AllReduce,
# base vectors via bf16 AllGather, sims columns concatenated on host.
# Inputs staged to bf16 on host (memory-bound kernel; rel-err budget 2e-2).

import numpy as np
import ml_dtypes

B, R, T, D = 256, 36, 64, 1024
NCORES = 8
CLOC = B // NCORES  # 32 local captions / images
NPAIR = CLOC // 2   # 16 caption pair-tiles (2*64 tokens = 128 partitions)
NIT = 11            # image tiles of 3 imgs (108 partitions); last tile 2 + pad
EPS_L2 = 1e-8
EPS_BN = 1e-5
LEAK = 0.1

_STATE = {}


def _build(tap=None):
    import concourse.bass as bass
    import concourse.bacc as bacc
    import concourse.tile as tile
    from concourse import mybir

    f32 = mybir.dt.float32
    bf16 = mybir.dt.bfloat16
    AF = mybir.ActivationFunctionType
    ALU = mybir.AluOpType

    nc = bacc.Bacc(
        "TRN2",
        target_bir_lowering=False,
        debug=False,
        enable_asserts=True,
        num_devices=NCORES,
    )

    # ---- kernel I/O -----------------------------------------------------
    cap_in = nc.dram_tensor("cap", [NPAIR, 128, D], bf16, kind="ExternalInput").ap()
    img_in = nc.dram_tensor("img", [NIT, 108, D], bf16, kind="ExternalInput").ap()
    wm_in = nc.dram_tensor("wm2", [128, NPAIR, 64], bf16, kind="ExternalInput").ap()
    imo_in = nc.dram_tensor("imones", [108, NIT, 32], bf16, kind="ExternalInput").ap()
    fc_in = nc.dram_tensor("fcT", [2, 8, 128, 1024], bf16, kind="ExternalInput").ap()
    fcb_in = nc.dram_tensor("fcb", [128, 16], f32, kind="ExternalInput").ap()
    bnf_in = nc.dram_tensor("bnF", [128, 16], f32, kind="ExternalInput").ap()
    cst_in = nc.dram_tensor("consts", [128, 8], f32, kind="ExternalInput").ap()
    onesr_in = nc.dram_tensor("ones_row", [1, 128], f32, kind="ExternalInput").ap()
    id_in = nc.dram_tensor("ident", [32, 32], f32, kind="ExternalInput").ap()
    out = nc.dram_tensor("out", [B, CLOC], f32, kind="ExternalOutput").ap()

    def tap_point(name, ap):
        if tap == name:
            shape = [ap.shape[0], int(np.prod(ap.shape[1:]))]
            dbg = nc.dram_tensor("dbg", shape, ap.dtype, kind="ExternalOutput").ap()
            nc.sync.dma_start(out=dbg[:, :], in_=ap)

    RG = [list(range(NCORES))]

    with tile.TileContext(nc) as tc:
        with (
            tc.tile_pool(name="dram", bufs=1, space="DRAM") as dpool,
            tc.tile_pool(name="io", bufs=4) as io,       # streaming input tiles
            tc.tile_pool(name="work", bufs=3) as work,   # relu/square working tiles
            tc.tile_pool(name="sb1", bufs=1) as sb1,     # long-lived single tensors
            tc.tile_pool(name="psA", bufs=1, space="PSUM") as psA,   # 4-bank accum
            tc.tile_pool(name="psS", bufs=4, space="PSUM") as psS,   # 1-bank smalls
        ):
            # ---- DRAM bounce buffers for collectives ----
            ar_in = dpool.tile([128, 16], f32)
            ar_out = dpool.tile([128, 16], f32, addr_space="Shared")
            ag_in = dpool.tile([128, 256], bf16)
            ag_out = dpool.tile([1024, 256], bf16, addr_space="Shared")

            # ---- constants ----
            csts = sb1.tile([128, 8], f32)
            nc.gpsimd.dma_start(out=csts[:], in_=cst_in[:, :])
            ones_row = sb1.tile([1, 128], f32)
            nc.gpsimd.dma_start(out=ones_row[:], in_=onesr_in[:, :])
            ident = sb1.tile([32, 32], f32)
            nc.gpsimd.dma_start(out=ident[:], in_=id_in[:, :])
            wm_sb = sb1.tile([128, NPAIR, 64], bf16)
            nc.gpsimd.dma_start(out=wm_sb[:], in_=wm_in[:, :, :])
            imo_sb = sb1.tile([108, NIT, 32], bf16)
            nc.gpsimd.dma_start(out=imo_sb[:], in_=imo_in[:, :, :])
            fcb_sb = sb1.tile([128, 16], f32)
            nc.gpsimd.dma_start(out=fcb_sb[:], in_=fcb_in[:, :])
            bnf_sb = sb1.tile([128, 16], f32)
            nc.gpsimd.dma_start(out=bnf_sb[:], in_=bnf_in[:, :])

            ones128 = csts[:, 5:6]  # ones column (128,1)

            def t_fold(dst_folded, src_sb, ncol=8):
                """(32, ncol*128) SBUF -> folded (128, ncol*32) via PE transpose."""
                for j in range(ncol):
                    pt = psS.tile([128, 32], f32, tag="sm")
                    nc.tensor.transpose(
                        pt[:], src_sb[:, j * 128 : (j + 1) * 128], ident[:]
                    )
                    nc.scalar.copy(dst_folded[:, j * 32 : (j + 1) * 32], pt[:])

            # ================= image pipeline (first: feeds collectives) ====
            # img DMAs on sync queue; cap DMAs on scalar queue (separate FIFOs)
            img_acc = psA.tile([32, 2048], f32, tag="acc")  # [s1 | s2]
            for p in range(NIT):
                xt = io.tile([108, D], bf16, tag="io")
                nc.sync.dma_start(out=xt[:], in_=img_in[p, :, :])
                yt = work.tile([108, D], bf16, tag="y")
                nc.vector.scalar_tensor_tensor(
                    yt[:], xt[:], LEAK, xt[:], ALU.mult, ALU.max
                )
                y2 = work.tile([108, D], bf16, tag="y2")
                nc.scalar.activation(y2[:], yt[:], AF.Square)
                st, sp = (p == 0), (p == NIT - 1)
                for h in range(2):
                    sl = slice(512 * h, 512 * (h + 1))
                    nc.tensor.matmul(
                        img_acc[:, sl],
                        lhsT=imo_sb[:, p, :],
                        rhs=yt[:, sl],
                        start=st,
                        stop=sp,
                    )
                    sl2 = slice(1024 + 512 * h, 1024 + 512 * (h + 1))
                    nc.tensor.matmul(
                        img_acc[:, sl2],
                        lhsT=imo_sb[:, p, :],
                        rhs=y2[:, sl],
                        start=st,
                        stop=sp,
                    )

            s1_sb = work.tile([32, D], f32, tag="cm1")
            nc.scalar.copy(s1_sb[:], img_acc[0:32, 0:1024])
            s2_sb = work.tile([32, D], f32, tag="cm2")
            nc.scalar.copy(s2_sb[:], img_acc[0:32, 1024:2048])
            tap_point("s1", s1_sb[:])
            tap_point("s2", s2_sb[:])

            s1T = sb1.tile([128, 256], f32)
            t_fold(s1T, s1_sb)
            s2T = sb1.tile([128, 256], f32)
            t_fold(s2T, s2_sb)

            # folded per-(b,d) norm math: n = sqrt(s2)+eps, w = 1/n
            nT = work.tile([128, 256], f32, tag="f1")
            nc.scalar.activation(nT[:], s2T[:], AF.Sqrt)
            nc.vector.tensor_scalar_add(nT[:], nT[:], EPS_L2)
            wT = work.tile([128, 256], f32, tag="f2")
            nc.vector.reciprocal(wT[:], nT[:])
            s1nT = sb1.tile([128, 256], f32)  # sum_r ytilde (folded d-major)
            nc.vector.tensor_mul(s1nT[:], s1T[:], wT[:])
            # s2n = ((n-eps)/n)^2 = (1 - eps*w)^2
            tT = work.tile([128, 256], f32, tag="f3")
            nc.vector.tensor_scalar(tT[:], wT[:], -EPS_L2, 1.0, ALU.mult, ALU.add)
            s2nT = work.tile([128, 256], f32, tag="f4")
            nc.vector.tensor_mul(s2nT[:], tT[:], tT[:])
            tap_point("s1nT", s1nT[:])

            # partial batch sums over local b (free axis within folded chunks)
            stat = work.tile([128, 16], f32, tag="st")
            nc.vector.reduce_sum(
                stat[:, 0:8],
                s1nT[:].rearrange("p (j c) -> p j c", j=8),
                axis=mybir.AxisListType.X,
            )
            nc.vector.reduce_sum(
                stat[:, 8:16],
                s2nT[:].rearrange("p (j c) -> p j c", j=8),
                axis=mybir.AxisListType.X,
            )
            nc.gpsimd.dma_start(out=ar_in[:], in_=stat[:])
            nc.gpsimd.collective_compute(
                "AllReduce",
                ALU.add,
                replica_groups=RG,
                ins=[ar_in[:]],
                outs=[ar_out[:]],
            )
            statg = work.tile([128, 16], f32, tag="stg")
            nc.gpsimd.dma_start(out=statg[:], in_=ar_out[:])
            tap_point("statg", statg[:])

            # BN stats (folded (128,8) columns): mu, var, g
            inv_br = 1.0 / (B * R)
            muF = work.tile([128, 8], f32, tag="s1f")
            nc.vector.tensor_scalar_mul(muF[:], statg[:, 0:8], inv_br)
            varF = work.tile([128, 8], f32, tag="s2f")
            nc.vector.tensor_scalar_mul(varF[:], statg[:, 8:16], inv_br)
            mu2F = work.tile([128, 8], f32, tag="s3f")
            nc.vector.tensor_mul(mu2F[:], muF[:], muF[:])
            nc.vector.tensor_sub(varF[:], varF[:], mu2F[:])
            nc.vector.tensor_scalar_add(varF[:], varF[:], EPS_BN)
            sqF = work.tile([128, 8], f32, tag="s4f")
            nc.scalar.activation(sqF[:], varF[:], AF.Sqrt)
            gF = work.tile([128, 8], f32, tag="s5f")
            nc.vector.reciprocal(gF[:], sqF[:])
            nc.vector.tensor_mul(gF[:], gF[:], bnf_sb[:, 0:8])  # g = bn_w*rsqrt
            GF = work.tile([128, 8], f32, tag="s6f")
            nc.vector.tensor_scalar_mul(GF[:], gF[:], 1.0 / R)  # G = g/R
            HF = work.tile([128, 8], f32, tag="s7f")
            nc.vector.tensor_mul(HF[:], muF[:], gF[:])
            nc.vector.tensor_sub(HF[:], HF[:], bnf_sb[:, 8:16])  # H = mu*g - bn_b

            # baseT folded (bf16 for AllGather): base = s1n*G - H per chunk
            baseT = sb1.tile([128, 256], bf16)
            for j in range(8):
                nc.vector.tensor_scalar(
                    baseT[:, j * 32 : (j + 1) * 32],
                    s1nT[:, j * 32 : (j + 1) * 32],
                    GF[:, j : j + 1],
                    HF[:, j : j + 1],
                    ALU.mult,
                    ALU.subtract,
                )
            tap_point("baseT", baseT[:])
            nc.gpsimd.dma_start(out=ag_in[:], in_=baseT[:])
            nc.gpsimd.collective_compute(
                "AllGather",
                ALU.bypass,
                replica_groups=RG,
                ins=[ag_in[:]],
                outs=[ag_out[:]],
            )

            # ================= caption pipeline =============================
            cap_acc = psA.tile([32, 2048], f32, tag="acc")  # [wsum | sumsq]
            for p in range(NPAIR):
                xt = io.tile([128, D], bf16, tag="io")
                nc.scalar.dma_start(out=xt[:], in_=cap_in[p, :, :])
                yt = work.tile([128, D], bf16, tag="y")
                nc.vector.scalar_tensor_tensor(
                    yt[:], xt[:], LEAK, xt[:], ALU.mult, ALU.max
                )
                y2 = work.tile([128, D], bf16, tag="y2")
                nc.scalar.activation(y2[:], yt[:], AF.Square)
                st, sp = (p == 0), (p == NPAIR - 1)
                for h in range(2):
                    sl = slice(512 * h, 512 * (h + 1))
                    nc.tensor.matmul(
                        cap_acc[:, sl],
                        lhsT=wm_sb[:, p, 0:32],
                        rhs=yt[:, sl],
                        start=st,
                        stop=sp,
                    )
                    sl2 = slice(1024 + 512 * h, 1024 + 512 * (h + 1))
                    nc.tensor.matmul(
                        cap_acc[:, sl2],
                        lhsT=wm_sb[:, p, 32:64],
                        rhs=y2[:, sl],
                        start=st,
                        stop=sp,
                    )

            ws_sb = work.tile([32, D], f32, tag="cm1")
            nc.scalar.copy(ws_sb[:], cap_acc[0:32, 0:1024])
            sq_sb = work.tile([32, D], f32, tag="cm2")
            nc.scalar.copy(sq_sb[:], cap_acc[0:32, 1024:2048])
            tap_point("ws", ws_sb[:])
            tap_point("sq", sq_sb[:])
            wsT = sb1.tile([128, 256], f32)
            t_fold(wsT, ws_sb)
            sqT = sb1.tile([128, 256], f32)
            t_fold(sqT, sq_sb)

            # cap_mean = wsum / (sqrt(sumsq)+eps)   (folded)
            cnT = work.tile([128, 256], f32, tag="f1")
            nc.scalar.activation(cnT[:], sqT[:], AF.Sqrt)
            nc.vector.tensor_scalar_add(cnT[:], cnT[:], EPS_L2)
            ciT = work.tile([128, 256], f32, tag="f2")
            nc.vector.reciprocal(ciT[:], cnT[:])
            cmT = sb1.tile([128, 256], f32)
            nc.vector.tensor_mul(cmT[:], wsT[:], ciT[:])
            tap_point("cmT", cmT[:])
            cmTb = sb1.tile([128, 256], bf16)
            nc.vector.tensor_copy(cmTb[:], cmT[:])

            # ================= FC: alphas/betas (d-major folded) ============
            alT = sb1.tile([128, 256], f32)
            beT = sb1.tile([128, 256], f32)
            abT = [alT, beT]
            for a in range(2):
                for j in range(8):
                    fw = io.tile([128, 1024], bf16, tag="fw")
                    nc.scalar.dma_start(out=fw[:], in_=fc_in[a, j, :, :])
                    ps = psS.tile([128, 32], f32, tag="sm")
                    for i in range(8):
                        nc.tensor.matmul(
                            ps[:],
                            lhsT=fw[:, i * 128 : (i + 1) * 128],
                            rhs=cmTb[:, i * 32 : (i + 1) * 32],
                            start=(i == 0),
                            stop=(i == 7),
                        )
                    nc.vector.tensor_scalar_add(
                        abT[a][:, j * 32 : (j + 1) * 32],
                        ps[:],
                        fcb_sb[:, a * 8 + j : a * 8 + j + 1],
                    )
            tap_point("alT", alT[:])
            tap_point("beT", beT[:])

            # ================= per-caption folded products ==================
            na_rhs = sb1.tile([128, 8, 64], bf16)  # [:,i,0:32]=uT [:,i,32:64]=2ab
            a2T = sb1.tile([128, 256], bf16)
            bcmT = work.tile([128, 256], f32, tag="f3")
            b2T = work.tile([128, 256], f32, tag="f4")
            cm2T = work.tile([128, 256], f32, tag="f5")
            nc.vector.tensor_mul(a2T[:], alT[:], alT[:])
            nc.vector.tensor_mul(bcmT[:], beT[:], cmT[:])
            nc.vector.tensor_mul(b2T[:], beT[:], beT[:])
            nc.vector.tensor_mul(cm2T[:], cmT[:], cmT[:])
            for i in range(8):
                sl = slice(i * 32, (i + 1) * 32)
                nc.vector.tensor_mul(na_rhs[:, i, 0:32], alT[:, sl], cmT[:, sl])
                tmp = na_rhs[:, i, 32:64]
                nc.vector.tensor_mul(tmp, alT[:, sl], beT[:, sl])
                nc.vector.tensor_add(tmp, tmp, tmp)

            # row reductions: cb, q3, nrm2 (each own PSUM bank!)
            rows_cb = psS.tile([1, 32], f32, tag="sm")
            rows_q3 = psS.tile([1, 32], f32, tag="sm")
            rows_n2 = psS.tile([1, 32], f32, tag="sm")
            for i in range(8):
                sl = slice(i * 32, (i + 1) * 32)
                st, sp = (i == 0), (i == 7)
                nc.tensor.matmul(
                    rows_cb[:], lhsT=ones128, rhs=bcmT[:, sl], start=st, stop=sp
                )
                nc.tensor.matmul(
                    rows_q3[:], lhsT=ones128, rhs=b2T[:, sl], start=st, stop=sp
                )
                nc.tensor.matmul(
                    rows_n2[:], lhsT=ones128, rhs=cm2T[:, sl], start=st, stop=sp
                )
            rows_sb = sb1.tile([1, 96], f32)
            nc.scalar.copy(rows_sb[:, 0:32], rows_cb[:])
            nc.scalar.copy(rows_sb[:, 32:64], rows_q3[:])
            nc.scalar.copy(rows_sb[:, 64:96], rows_n2[:])
            tap_point("rows", rows_sb[:])
            # invn = 1/(sqrt(nrm2)+eps)
            invn = sb1.tile([1, 32], f32)
            nc.scalar.activation(invn[:], rows_sb[:, 64:96], AF.Sqrt)
            nc.vector.tensor_scalar_add(invn[:], invn[:], EPS_L2)
            nc.vector.reciprocal(invn[:], invn[:])
            # broadcast invn across partitions via K=1 matmul
            inb_ps = psS.tile([128, 32], f32, tag="sm")
            nc.tensor.matmul(
                inb_ps[:], lhsT=ones_row[:], rhs=invn[:], start=True, stop=True
            )
            invn_sb = sb1.tile([128, 32], f32)
            nc.scalar.copy(invn_sb[:], inb_ps[:])

            # ================= sims matmuls + epilogue ======================
            na0 = psS.tile([128, 64], f32, tag="sm")
            na1 = psS.tile([128, 64], f32, tag="sm")
            nas = [na0, na1]
            for i in range(8):
                bT = io.tile([128, 256], bf16, tag="bT")
                # gather chunk i of global baseT from AG output (bf16)
                src = bass.AP(
                    tensor=ag_out.tensor,
                    offset=ag_out.offset + 32 * i,
                    ap=[[256, 128], [128 * 256, 8], [1, 32]],
                )
                nc.sync.dma_start(out=bT[:], in_=src)
                b2 = io.tile([128, 256], bf16, tag="b2")
                nc.vector.tensor_mul(b2[:], bT[:], bT[:])
                for m in range(2):
                    bsl = slice(m * 128, (m + 1) * 128)
                    nc.tensor.matmul(
                        nas[m][:],
                        lhsT=bT[:, bsl],
                        rhs=na_rhs[:, i, :],
                        start=(i == 0),
                        stop=False,
                    )
                    nc.tensor.matmul(
                        nas[m][:, 32:64],
                        lhsT=b2[:, bsl],
                        rhs=a2T[:, i * 32 : (i + 1) * 32],
                        start=False,
                        stop=False,
                        skip_group_check=True,
                    )
            for m in range(2):
                na = nas[m]
                # num += cb, den2 += q3 broadcast over b: K=1 matmul
                nc.tensor.matmul(
                    na[:],
                    lhsT=ones_row[:],
                    rhs=rows_sb[:, 0:64],
                    start=False,
                    stop=True,
                )
                den = work.tile([128, 32], f32, tag="ep1")
                nc.scalar.activation(den[:], na[:, 32:64], AF.Sqrt)
                nc.vector.tensor_scalar_add(den[:], den[:], EPS_L2)
                rec = work.tile([128, 32], f32, tag="ep2")
                nc.vector.reciprocal(rec[:], den[:])
                sims = work.tile([128, 32], f32, tag="ep3")
                nc.vector.tensor_mul(sims[:], na[:, 0:32], rec[:])
                nc.vector.tensor_mul(sims[:], sims[:], invn_sb[:])
                nc.sync.dma_start(out=out[m * 128 : (m + 1) * 128, :], in_=sims[:])

    nc.compile()
    return nc


def _prep_inputs(img_embed, cap_embed, lens, fc_w, fc_b, bn_w, bn_b):
    f32 = np.float32
    bf16 = ml_dtypes.bfloat16
    lens_f = lens.astype(f32)
    wmask_w = (np.arange(T)[None, :] < lens[:, None]).astype(f32) / lens_f[:, None]

    # constants (same all cores)
    consts = np.zeros((128, 8), f32)
    consts[:, 5] = 1.0
    ones_row = np.ones((1, 128), f32)
    ident = np.eye(32, dtype=f32)

    # image block-diag ones lhsT per tile: col 3t+j = 1 on rows 36j:36(j+1)
    imones = np.zeros((108, NIT, 32), f32)
    for t in range(NIT):
        for j in range(min(3, CLOC - 3 * t)):
            imones[36 * j : 36 * (j + 1), t, 3 * t + j] = 1.0
    imones = imones.astype(bf16)

    # fcT[a, j, kk, i*128+dd] = fc_w[2*(128j+dd)+a, 128i+kk]
    A = fc_w.reshape(1024, 2, 1024).transpose(1, 0, 2)  # (a, dout, k)
    A5 = A.reshape(2, 8, 128, 8, 128)  # (a, j, dd, i, kk)
    fcT = np.ascontiguousarray(A5.transpose(0, 1, 4, 3, 2)).reshape(2, 8, 128, 1024)
    fcT = fcT.astype(bf16)
    # fcb[dd, a*8+j] = fc_b[2*(128j+dd)+a]
    fcb = np.ascontiguousarray(
        fc_b.reshape(8, 128, 2).transpose(1, 2, 0)
    ).reshape(128, 16).astype(f32)
    # bnF[p, j]=bn_w[128j+p], bnF[p, 8+j]=bn_b[128j+p]
    bnF = np.concatenate(
        [bn_w.reshape(8, 128).T, bn_b.reshape(8, 128).T], axis=1
    ).astype(f32)

    in_maps = []
    for k in range(NCORES):
        s = slice(CLOC * k, CLOC * (k + 1))
        cap_k = np.ascontiguousarray(cap_embed[s]).reshape(NPAIR, 128, D)
        img_k = np.zeros((NIT, 108, D), f32)
        imgs = img_embed[s]
        for t in range(NIT):
            n = min(3, CLOC - 3 * t)
            img_k[t, : 36 * n, :] = imgs[3 * t : 3 * t + n].reshape(36 * n, D)
        wm = np.zeros((128, NPAIR, 64), f32)
        for p in range(NPAIR):
            for c in range(2):
                rows = slice(64 * c, 64 * (c + 1))
                wm[rows, p, 2 * p + c] = wmask_w[CLOC * k + 2 * p + c]
                wm[rows, p, 32 + 2 * p + c] = 1.0
        in_maps.append(
            {
                "cap": cap_k.astype(bf16),
                "img": img_k.astype(bf16),
                "wm2": wm.astype(bf16),
                "imones": imones,
                "fcT": fcT,
                "fcb": fcb,
                "bnF": bnF,
                "consts": consts,
                "ones_row": ones_row,
                "ident": ident,
            }
        )
    return in_maps


def run(inputs, trace=False, tap=None, **kw):
    from concourse import bass_utils

    key = ("nc", tap)
    if key not in _STATE:
        _STATE[key] = _build(tap)
    res = bass_utils.run_bass_kernel_spmd(
        _STATE[key], in_maps := _prep_inputs(**inputs), core_ids=list(range(NCORES)),
        trace=trace, **kw
    )
    sims = np.concatenate([res.results[k]["out"] for k in range(NCORES)], axis=1)
    return sims.astype(np.float32), res


def kernel(**inputs):
    sims, _ = run(inputs, trace=False)
    return sims
